# revision 1
# baseline (speedup 1.0000x reference)
"""Causal self-attention on 8 NeuronCores (Trainium2, Bass/Tile).

Problem: B=4, T=2048, C=1024, H=16 heads, HD=64, fp32 in/out.
    qkv = x @ Wqkv + bqkv ; causal softmax attention ; y @ Wproj + bproj

Sharding (Megatron-style): 8 cores = 4 batches x 2 head-groups.
Core c handles batch b = c//2 and head group g = c%2 (8 heads each).
Each core computes a partial output projection over its 512 head-dims;
the host sums the two partials per batch and adds bproj.

Design (measured via the TRN2 cost-model timeline; ~1.7x faster than
the fp32r baseline, 471us -> ~282us per core):
  * bf16 everywhere on SBUF (end-to-end rel err ~3.5e-3 vs the 2e-2
    gate); PSUM accumulation stays fp32. Halves DMA + SBUF footprint,
    full PE rate at any moving size (fp32r needs >=256), 2x DVE rate.
  * Phase-2 critical path is scores-matmul -> exp with nothing between:
    the causal triangle is applied AFTER exp, multiplying pt's diagonal
    128x128 sub-block by a 0/1 triangle on DVE (off the ACT path), and
    fully-masked columns are never computed at all (partial-width
    scores, exp, and A.V on diagonal blocks).
  * Software-pipelined emission: scores emitted one block ahead, and
    per-head-pair epilogues deferred one block, so PE's in-order queue
    never head-of-line blocks on exp or the reciprocal. QKV of chunk
    qc+1 and the output projection of chunk qc-1 are interleaved into
    chunk qc's attention loop to fill PE slack while ACT (exp) limits.
  * PSUM: ps_s 2x2 banks (scores only), ps_f 2x1 (qkv/proj/recip
    broadcast), ps_u 2x1 (A.V accumulators). Keeping scores in a
    dedicated pool decouples the exp pipeline from filler work.
  * DMAs are batched (one 3D-AP DMA per x half / weight / proj column
    group) because each DMA costs ~650ns of SP sequencer issue time;
    ramp weights go on the Activation HWDGE queue to overlap the SP
    queue. x buffers are deep enough (6) to never stall reuse.
  * Softmax denominators ride the A.V matmul as a 65th "ones" column of
    V; the divide broadcasts 1/sumexp with a K=1 matmul, bounced via
    SBUF because DVE may read only one PSUM operand per instruction.
"""
import sys

for _p in ("/opt/trn_rl_repo",):
    if _p not in sys.path:
        sys.path.append(_p)

import numpy as np

B, T, C = 4, 2048, 1024
H, HD = 16, 64
N_CORES = 8
G_HEADS = 8            # heads per core (one group)
G_FEAT = G_HEADS * HD  # 512 feature dims per group
VW = HD + 1            # V block stride per head (64 values + ones col)

TOKC = 512             # token chunk == query chunk
QC = 512
n_cc = C // 128        # 8 contraction chunks
n_hp = G_HEADS // 2    # 4 head pairs
n_qc = T // QC         # 4 chunks

_CACHE = {}


def _build_program():
    import contextlib
    import concourse.tile as tile
    from concourse import bacc, mybir

    F32 = mybir.dt.float32
    BF16 = mybir.dt.bfloat16
    Exp = mybir.ActivationFunctionType.Exp

    nc = bacc.Bacc("TRN2", target_bir_lowering=False, debug=False,
                   num_devices=N_CORES)

    xT_d = nc.dram_tensor("xT", [C, T], BF16, kind="ExternalInput").ap()
    wqk_d = nc.dram_tensor("wqk", [C, 2 * G_FEAT], BF16, kind="ExternalInput").ap()
    wv_d = nc.dram_tensor("wv", [C, G_FEAT], BF16, kind="ExternalInput").ap()
    bqk_d = nc.dram_tensor("bqk", [2 * G_FEAT], F32, kind="ExternalInput").ap()
    bv_d = nc.dram_tensor("bv", [1, G_FEAT], BF16, kind="ExternalInput").ap()
    wp_d = nc.dram_tensor("wp", [G_FEAT, C], BF16, kind="ExternalInput").ap()
    part_d = nc.dram_tensor("part", [T, C], BF16, kind="ExternalOutput").ap()

    with tile.TileContext(nc) as tc, contextlib.ExitStack() as ctx:
        const = ctx.enter_context(tc.tile_pool(name="const", bufs=1))
        wpool = ctx.enter_context(tc.tile_pool(name="weights", bufs=1))
        big = ctx.enter_context(tc.tile_pool(name="big", bufs=1))
        xpool = ctx.enter_context(tc.tile_pool(name="xT", bufs=6))
        ytpool = ctx.enter_context(tc.tile_pool(name="yT", bufs=2))
        ptpool = ctx.enter_context(tc.tile_pool(name="pt", bufs=4))
        ybpool = ctx.enter_context(tc.tile_pool(name="ybs", bufs=2))
        rcpool = ctx.enter_context(tc.tile_pool(name="recip", bufs=4))
        otpool = ctx.enter_context(tc.tile_pool(name="ot", bufs=2))
        ps_s = ctx.enter_context(
            tc.tile_pool(name="ps_s", bufs=2, space="PSUM"))
        ps_f = ctx.enter_context(
            tc.tile_pool(name="ps_f", bufs=2, space="PSUM"))
        ps_u = ctx.enter_context(
            tc.tile_pool(name="ps_u", bufs=2, space="PSUM"))

        ctx.enter_context(nc.allow_low_precision(
            reason="bf16 kernel end-to-end; rel-err gate is 2e-2"))

        # ---- constants ----
        ones_f32 = const.tile([128, 128], F32)
        nc.vector.memset(ones_f32[:], 1.0)
        ones_row = const.tile([1, 128], BF16)
        nc.vector.tensor_copy(ones_row[:], ones_f32[0:1, :])
        # 0/1 causal triangle: 1 where col >= row (valid), 0 above
        tri_f32 = const.tile([128, 128], F32)
        nc.vector.memset(tri_f32[:], 1.0)
        nc.gpsimd.affine_select(
            out=tri_f32[:], in_=tri_f32[:],
            compare_op=mybir.AluOpType.is_ge, fill=0.0, base=0,
            pattern=[[1, 128]], channel_multiplier=-1)
        tri01 = const.tile([128, 128], BF16)
        nc.vector.tensor_copy(tri01[:], tri_f32[:])

        # ---- resident weights ----
        # single batched DMAs on the Activation HWDGE queue so they overlap
        # the x-chunk DMAs on the SP queue during the ramp.
        wqk_sb = wpool.tile([128, n_cc * 2 * G_FEAT], BF16)
        nc.scalar.dma_start(
            wqk_sb[:].rearrange("p (c w) -> p c w", c=n_cc),
            wqk_d.rearrange("(c p) w -> p c w", p=128))
        bqk_sb = wpool.tile([128, 8], F32)
        nc.scalar.dma_start(bqk_sb[:], bqk_d.rearrange("(f p) -> p f", p=128))
        bv_sb = wpool.tile([1, G_FEAT], BF16)
        nc.scalar.dma_start(bv_sb[:], bv_d[:])

        # ---- big activations ----
        qt_sb = big.tile([128, n_hp * T], BF16)  # [feat, tok] head-pair major
        kt_sb = big.tile([128, n_hp * T], BF16)
        n_tb = T // 128
        v_sb = big.tile([128, n_tb * G_HEADS * VW], BF16)
        nc.vector.memset(
            v_sb[:].rearrange("p (t w) -> p t w", w=VW)[:, :, HD:HD + 1], 1.0)

        half = n_cc // 2

        def dma_x(qc):
            xts = []
            for hf in range(2):
                xt = xpool.tile([128, half * TOKC], BF16, tag="xT", name="xt")
                nc.sync.dma_start(
                    xt[:].rearrange("p (c t) -> p c t", c=half),
                    xT_d[hf * half * 128:(hf + 1) * half * 128,
                         qc * TOKC:(qc + 1) * TOKC]
                    .rearrange("(c p) t -> p c t", p=128))
                xts.append(xt)
            return xts

        def p1_qk_unit(qc, xts, f):
            """One Q^T/K^T feature block (128 feats x 512 toks)."""
            pqk = ps_f.tile([128, TOKC], F32, tag="f", name="pqk")
            for cc in range(n_cc):
                nc.tensor.matmul(
                    pqk[:],
                    wqk_sb[:, cc * 2 * G_FEAT + f * 128:
                           cc * 2 * G_FEAT + f * 128 + 128],
                    xts[cc // half][:, (cc % half) * TOKC:
                                    (cc % half + 1) * TOKC],
                    start=(cc == 0), stop=(cc == n_cc - 1))
            dst = qt_sb if f < 4 else kt_sb
            fb = f % 4
            nc.vector.tensor_scalar_add(
                dst[:, fb * T + qc * TOKC: fb * T + (qc + 1) * TOKC],
                pqk[:], bqk_sb[:, f:f + 1])

        def p1_v_unit(qc, xts, tb):
            """One V token block (128 toks x 512 feats) into VW layout."""
            tbg = qc * (TOKC // 128) + tb
            pv = ps_f.tile([128, G_FEAT], F32, tag="f", name="pv")
            for cc in range(n_cc):
                nc.tensor.matmul(
                    pv[:],
                    xts[cc // half][:, (cc % half) * TOKC + tb * 128:
                                    (cc % half) * TOKC + tb * 128 + 128],
                    wv_sb[:, cc * G_FEAT:(cc + 1) * G_FEAT],
                    start=(cc == 0), stop=False)
            nc.tensor.matmul(pv[:], ones_row[:], bv_sb[:],
                             start=False, stop=True)
            nc.vector.tensor_copy(
                v_sb[:, tbg * G_HEADS * VW:(tbg + 1) * G_HEADS * VW]
                .rearrange("p (h w) -> p h w", w=VW)[:, :, 0:HD],
                pv[:].rearrange("p (h w) -> p h w", w=HD))

        def p1_units(qc, xts):
            for f in range(8):
                yield lambda f=f: p1_qk_unit(qc, xts, f)
            for tb in range(TOKC // 128):
                yield lambda tb=tb: p1_v_unit(qc, xts, tb)

        def p1_units_first(qc, xts):
            """What chunk qc's first head-pair needs: its Q/K feature
            blocks (f=0 q, f=4 k) and all V blocks (diag A.V)."""
            for f in (0, 4):
                yield lambda f=f: p1_qk_unit(qc, xts, f)
            for tb in range(TOKC // 128):
                yield lambda tb=tb: p1_v_unit(qc, xts, tb)

        def p1_units_rest(qc, xts):
            """Head-pair hp needs f=hp/f=4+hp only once its own blocks
            start, so these can run inside chunk qc's early attention."""
            for f in (1, 5, 2, 6, 3, 7):
                yield lambda f=f: p1_qk_unit(qc, xts, f)

        def p3_unit(qc, yt, ot, n, tb):
            """One output-projection block of chunk qc; DMA once per n."""
            po = ps_f.tile([128, 512], F32, tag="f", name="po")
            for hp in range(n_hp):
                nc.tensor.matmul(
                    po[:],
                    yt[:, hp * QC + tb * 128: hp * QC + tb * 128 + 128],
                    wp_sb[:, hp * C + n * 512: hp * C + n * 512 + 512],
                    start=(hp == 0), stop=(hp == n_hp - 1))
            nc.vector.tensor_copy(ot[:, tb * 512:(tb + 1) * 512], po[:])
            if tb == QC // 128 - 1:
                nc.sync.dma_start(
                    part_d[qc * QC:(qc + 1) * QC, n * 512:(n + 1) * 512]
                    .rearrange("(b p) w -> p b w", p=128),
                    ot[:].rearrange("p (b w) -> p b w", b=QC // 128))

        def p3_units(qc, yt):
            for n in range(C // 512):
                ot = otpool.tile([128, (QC // 128) * 512], BF16, tag="ot",
                                 name="ot")
                for tb in range(QC // 128):
                    yield lambda n=n, tb=tb, ot=ot: p3_unit(qc, yt, ot, n, tb)

        # ---- phase 1 for chunk 0, then remaining weights ----
        xts0 = dma_x(0)
        wv_sb = wpool.tile([128, n_cc * G_FEAT], BF16)
        nc.scalar.dma_start(
            wv_sb[:].rearrange("p (c w) -> p c w", c=n_cc),
            wv_d.rearrange("(c p) w -> p c w", p=128))
        wp_sb = wpool.tile([128, 4 * C], BF16)
        nc.scalar.dma_start(
            wp_sb[:].rearrange("p (c w) -> p c w", c=4),
            wp_d.rearrange("(c p) w -> p c w", p=128))
        for u in p1_units(0, xts0):
            u()

        # ============ attention per query chunk ============
        yts = {}
        deferred = []
        for qc in range(n_qc):
            nkb = 4 * qc + 4
            yt = ytpool.tile([128, n_hp * QC], BF16, tag="yT", name="yt")
            yts[qc] = yt
            blocks = [(hp, ki) for hp in range(n_hp) for ki in range(nkb)]

            # work to interleave into this chunk's attention blocks:
            # this chunk's own deferred QKV first (hp1..3 need it soon),
            # then the next chunk's lead QKV units, then proj of qc-1.
            fillers = deferred
            deferred = []
            if qc + 1 < n_qc:
                xts_n = dma_x(qc + 1)
                fillers.extend(p1_units_first(qc + 1, xts_n))
                deferred = list(p1_units_rest(qc + 1, xts_n))
            if qc - 1 >= 0:
                fillers.extend(p3_units(qc - 1, yts[qc - 1]))
            n_fill = len(fillers)


            s_tiles = {}
            ua, ub = {}, {}

            def w0_of(ki, qc=qc):
                j = ki - 4 * qc
                return 128 * j if j > 0 else 0

            def emit_scores(blk, qc=qc):
                hp, ki = blk
                w0 = w0_of(ki)
                s = ps_s.tile([128, 2 * QC], F32, tag="s", name="s")
                qa = qt_sb[0:64, hp * T + qc * QC + w0: hp * T + (qc + 1) * QC]
                qb = qt_sb[64:128, hp * T + qc * QC + w0: hp * T + (qc + 1) * QC]
                nc.tensor.matmul(
                    s[:, w0:QC],
                    kt_sb[0:64, hp * T + ki * 128: hp * T + ki * 128 + 128],
                    qa, start=True, stop=True, tile_position=(0, 0))
                nc.tensor.matmul(
                    s[:, QC + w0:2 * QC],
                    kt_sb[64:128, hp * T + ki * 128: hp * T + ki * 128 + 128],
                    qb, start=True, stop=True, tile_position=(64, 0))
                s_tiles[blk] = s

            emit_scores(blocks[0])
            pending_epi = []
            for idx, blk in enumerate(blocks):
                if idx + 1 < len(blocks):
                    emit_scores(blocks[idx + 1])
                while pending_epi:
                    pending_epi.pop(0)()
                hp, ki = blk
                j = ki - 4 * qc
                w0 = w0_of(ki)
                s = s_tiles.pop(blk)
                pt = ptpool.tile([128, 2 * QC], BF16, tag="pt", name="pt")
                if w0 == 0:
                    nc.scalar.activation(pt[:], s[:], Exp,
                                         bias=0.0, scale=0.125)
                else:
                    sv = s[:].rearrange("p (h q) -> p h q", h=2)[:, :, w0:QC]
                    pv_ = pt[:].rearrange("p (h q) -> p h q", h=2)[:, :, w0:QC]
                    nc.scalar.activation(pv_, sv, Exp, bias=0.0, scale=0.125)
                if j >= 0:  # diagonal block: 0/1 triangle on the 128-col edge
                    nc.vector.tensor_mul(
                        pt[:, w0:w0 + 128], pt[:, w0:w0 + 128], tri01[:])
                    nc.vector.tensor_mul(
                        pt[:, QC + w0:QC + w0 + 128],
                        pt[:, QC + w0:QC + w0 + 128], tri01[:])
                if ki == 0:
                    ua[hp] = ps_u.tile([VW, QC], F32, tag="u", name="ua")
                    ub[hp] = ps_u.tile([VW, QC], F32, tag="u", name="ub")
                va = v_sb[:, (ki * G_HEADS + 2 * hp) * VW:
                          (ki * G_HEADS + 2 * hp) * VW + VW]
                vb = v_sb[:, (ki * G_HEADS + 2 * hp + 1) * VW:
                          (ki * G_HEADS + 2 * hp + 1) * VW + VW]
                last = (ki == nkb - 1)
                nc.tensor.matmul(ua[hp][:, w0:QC], va, pt[:, w0:QC],
                                 start=(ki == 0), stop=last)
                nc.tensor.matmul(ub[hp][:, w0:QC], vb, pt[:, QC + w0:2 * QC],
                                 start=(ki == 0), stop=last)
                if last:
                    # epilogue: divide by sumexp (psum row HD of ua/ub).
                    # reciprocal now (off PE); consumers deferred one block
                    # so PE does not head-of-line block on the recip. DVE
                    # reads at most ONE PSUM input, so the K=1 broadcast
                    # matmul result bounces through SBUF (rb).
                    rc = rcpool.tile([1, 2 * QC], BF16, tag="recip", name="rc")
                    nc.vector.reciprocal(rc[:, 0:QC], ua[hp][HD:HD + 1, :])
                    nc.vector.reciprocal(rc[:, QC:2 * QC],
                                         ub[hp][HD:HD + 1, :])

                    def epi(hp=hp, rc=rc):
                        r_a = ps_f.tile([64, QC], F32, tag="f", name="r_a")
                        r_b = ps_f.tile([64, QC], F32, tag="f", name="r_b")
                        nc.tensor.matmul(r_a[:], ones_row[:, 0:64],
                                         rc[:, 0:QC], start=True, stop=True)
                        nc.tensor.matmul(r_b[:], ones_row[:, 0:64],
                                         rc[:, QC:2 * QC],
                                         start=True, stop=True)
                        rb = ybpool.tile([64, 2 * QC], BF16, tag="rb",
                                         name="rb")
                        nc.vector.tensor_copy(rb[:, 0:QC], r_a[:])
                        nc.vector.tensor_copy(rb[:, QC:2 * QC], r_b[:])
                        nc.vector.tensor_mul(
                            yt[0:64, hp * QC:(hp + 1) * QC],
                            ua[hp][0:HD, :], rb[:, 0:QC])
                        yb = ybpool.tile([64, QC], BF16, tag="ybs", name="yb")
                        nc.vector.tensor_mul(yb[:], ub[hp][0:HD, :],
                                             rb[:, QC:2 * QC])
                        nc.sync.dma_start(
                            yt[64:128, hp * QC:(hp + 1) * QC], yb[:])
                    pending_epi.append(epi)
                # interleaved filler work (QKV of qc+1, proj of qc-1)
                want = ((idx + 1) * n_fill) // len(blocks)
                while fillers and n_fill - len(fillers) < want:
                    fillers.pop(0)()
            while pending_epi:
                pending_epi.pop(0)()
            while fillers:
                fillers.pop(0)()

        # final chunk's projection
        for u in p3_units(n_qc - 1, yts[n_qc - 1]):
            u()

    nc.compile()
    return nc


def _get_program():
    if "nc" not in _CACHE:
        _CACHE["nc"] = _build_program()
    return _CACHE["nc"]


def make_in_maps(x, Wqkv, bqkv, Wproj):
    """Shard full inputs into the 8 per-core input maps (bf16)."""
    from concourse import mybir
    bf16 = mybir.dt.np(mybir.dt.bfloat16)

    x = np.asarray(x, dtype=np.float32)
    Wqkv = np.asarray(Wqkv, dtype=np.float32)
    bqkv = np.asarray(bqkv, dtype=np.float32)
    Wproj = np.asarray(Wproj, dtype=np.float32)

    xT = [np.ascontiguousarray(x[b].T).astype(bf16) for b in range(B)]
    wqk, wv, bqk, bv, wp = [], [], [], [], []
    for g in range(2):
        qs, ks, vs = 512 * g, C + 512 * g, 2 * C + 512 * g
        wqk.append(np.ascontiguousarray(np.concatenate(
            [Wqkv[:, qs:qs + 512], Wqkv[:, ks:ks + 512]], axis=1)).astype(bf16))
        wv.append(np.ascontiguousarray(Wqkv[:, vs:vs + 512]).astype(bf16))
        bqk.append(np.ascontiguousarray(
            np.concatenate([bqkv[qs:qs + 512], bqkv[ks:ks + 512]])))
        bv.append(np.ascontiguousarray(
            bqkv[vs:vs + 512].reshape(1, -1)).astype(bf16))
        wp.append(np.ascontiguousarray(
            Wproj[512 * g:512 * g + 512, :]).astype(bf16))

    maps = []
    for c in range(N_CORES):
        b, g = c // 2, c % 2
        maps.append({"xT": xT[b], "wqk": wqk[g], "wv": wv[g],
                     "bqk": bqk[g], "bv": bv[g], "wp": wp[g]})
    return maps


def kernel(x, Wqkv, bqkv, Wproj, bproj):
    from concourse.bass_utils import run_bass_kernel_spmd

    nc = _get_program()
    in_maps = make_in_maps(x, Wqkv, bqkv, Wproj)
    res = run_bass_kernel_spmd(nc, in_maps, list(range(N_CORES)))
    bproj = np.asarray(bproj, dtype=np.float32)
    out = np.empty((B, T, C), dtype=np.float32)
    for b in range(B):
        out[b] = (res.results[2 * b]["part"].astype(np.float32)
                  + res.results[2 * b + 1]["part"].astype(np.float32) + bproj)
    return out

